# revision 1
# baseline (speedup 1.0000x reference)
"""Trainium2 Bass kernel for nn_CosineLoss: mean_i(1 - output[i, targets[i]]).

Strategy (data-parallel over the batch dim, 8 cores):
  - Core c owns rows [c*1024, (c+1)*1024) of `output` ([1024, 32000] f32 shard)
    plus flat element offsets idx[i] = i*32000 + targets[i] for its rows
    (int32, laid out [128, 8] in SBUF; descriptor address math in the SWDGE
    is integer, while on-device ALU adds go through an fp32 path that
    corrupts indices above 2^24).
  - On device: 8 indirect DMAs (128 descriptors each -- the HW unrolls one
    descriptor per dest partition row, so 128 scattered elements per
    instruction is the cap) gather the 1024 needed f32 elements from HBM
    (4 KB instead of 131 MB).
  - Raw bass (no TileContext): a hand-rolled semaphore graph avoids the
    tile-pool exit cleanup (drain + RANGE_CLEAR + barrier sandwich, ~2us of
    the measured window).
  - Split free-dim reduce 7/1: cols 0:7 reduce while gather 8 is still in
    flight; only a [128,1] add sits behind the last gather's completion.
    A [128,1] matmul against the framework's const-1.0 tile collapses
    partitions; the Activation engine copies the [1,1] PSUM scalar to SBUF
    and triggers the output DMA in engine order (no cross-engine hop).
  - Host sums the 8 partials and returns 1 - total/8192 as a () f32 array.
"""

import numpy as np

from concourse import bacc, bass, mybir
from concourse.bass_utils import run_bass_kernel_spmd

N = 8192
C = 32000
NCORES = 8
NL = N // NCORES  # 1024 rows per core
P = 128
F = NL // P  # 8 gathered elements per partition

_NC_CACHE = {}


def _build():
    # Bacc (not Bass): its compile() runs generate_event_semaphores, which
    # splits multi-sem waits -- walrus codegen allows 1 sync wait per inst.
    nc = bacc.Bacc("TRN2")
    x = nc.dram_tensor("x", [NL, C], mybir.dt.float32, kind="ExternalInput")
    idx = nc.dram_tensor("idx", [P, F], mybir.dt.int32, kind="ExternalInput")
    partial = nc.dram_tensor("partial", [1, 1], mybir.dt.float32, kind="ExternalOutput")

    idx_t = nc.alloc_sbuf_tensor("idx_t", [P, F], mybir.dt.int32)
    gath = nc.alloc_sbuf_tensor("gath", [P, F], mybir.dt.float32)
    red_a = nc.alloc_sbuf_tensor("red_a", [P, 1], mybir.dt.float32)
    red_f = nc.alloc_sbuf_tensor("red_f", [P, 1], mybir.dt.float32)
    res = nc.alloc_sbuf_tensor("res", [1, 1], mybir.dt.float32)
    acc = nc.alloc_psum_tensor("acc", [1, 1], mybir.dt.float32)

    # ones for the partition-reduce matmul: the framework preamble already
    # memsets a [128,1] f32 1.0 const tile, and every engine passes the
    # init barrier after it -- no extra memset or semaphore needed.
    ones = nc.const_aps.aps[(mybir.dt.float32, 1.0)]

    s_idx = nc.alloc_semaphore("s_idx")  # idx DMA completion (+16)
    s_ga = nc.alloc_semaphore("s_ga")  # gathers 1-7 DMA completions (+16 each)
    s_gb = nc.alloc_semaphore("s_gb")  # gather 8 DMA completion (+16)
    s_v = nc.alloc_semaphore("s_v")  # DVE progress
    s_mm = nc.alloc_semaphore("s_mm")  # matmul (HIGH pass) done
    s_out = nc.alloc_semaphore("s_out")  # walrus requires an update on every DMA

    nc.sync.dma_start(out=idx_t.ap(), in_=idx[:]).then_inc(s_idx, 16)

    # dummy early copy on the (idle) Activation engine: if walrus needs an
    # activation-table load for `copy`, it lands here, fully overlapped with
    # the idx DMA, instead of on the critical path before the real copy.
    actwarm = nc.alloc_sbuf_tensor("actwarm", [1, 1], mybir.dt.float32)
    nc.scalar.copy(out=actwarm.ap(), in_=nc.const_aps.aps[(mybir.dt.float32, 0.0)][0:1, :])

    # (A dummy SWDGE warmup gather was tried here to hide the first indirect
    # DMA's ~0.9us Q7 IRAM load behind the idx DMA; measured 24.6us vs 24.2us
    # without -- the ucode load only starts when the first DMA_INDIRECT
    # executes, so a warmup just pays it earlier plus its own gen time.)

    nc.gpsimd.wait_ge(s_idx, 16)
    for j in range(F):
        nc.gpsimd.indirect_dma_start(
            out=gath.ap()[:, j : j + 1],
            out_offset=None,
            in_=x[:],
            in_offset=bass.IndirectOffsetOnAxis(ap=idx_t.ap()[:, j : j + 1], axis=1),
        ).then_inc(s_ga if j < F - 1 else s_gb, 16)

    # 7/1 split: cols 0:7 reduce while gather 8 is still in flight; only a
    # single [P,1] add sits behind the last gather's completion semaphore.
    nc.vector.wait_ge(s_ga, 16 * (F - 1))
    nc.vector.tensor_reduce(
        out=red_a.ap(),
        in_=gath.ap()[:, 0 : F - 1],
        axis=mybir.AxisListType.X,
        op=mybir.AluOpType.add,
    ).then_inc(s_v, 1)
    nc.vector.wait_ge(s_gb, 16)
    nc.vector.scalar_tensor_tensor(
        out=red_f.ap(),
        in0=gath.ap()[:, F - 1 : F],
        scalar=0.0,
        in1=red_a.ap(),
        op0=mybir.AluOpType.add,
        op1=mybir.AluOpType.add,
    ).then_inc(s_v, 1)

    # partition-reduce via matmul with ones as lhsT: LDWEIGHTS(ones)
    # prefetches early (move_matmul_waits_to_ldweights hoists the waits);
    # only the rhs stream (red_f) sits on the critical path.
    nc.tensor.wait_ge(s_v, 2)
    nc.tensor.matmul(
        out=acc.ap(), lhsT=ones, rhs=red_f.ap(), start=True, stop=True
    ).then_inc(s_mm, 1)

    # PSUM->SBUF copy and the output DMA both on the Activation engine
    # (an HWDGE engine): engine-queue order replaces the copy->DMA
    # cross-engine semaphore hop.
    nc.scalar.wait_ge(s_mm, 1)
    nc.scalar.copy(out=res.ap(), in_=acc.ap())
    nc.scalar.dma_start(out=partial[:], in_=res.ap()).then_inc(s_out, 16)

    nc.compile()
    return nc


def _get_nc():
    if "nc" not in _NC_CACHE:
        _NC_CACHE["nc"] = _build()
    return _NC_CACHE["nc"]


def _shard(output, targets):
    xs = np.ascontiguousarray(
        output.reshape(NCORES, NL, C).astype(np.float32, copy=False)
    )
    flat = np.arange(NL, dtype=np.int32) * C + targets.reshape(NCORES, NL).astype(
        np.int32
    )
    return xs, np.ascontiguousarray(flat.reshape(NCORES, P, F))


def _run(output, targets, **kwargs):
    xs, idx = _shard(output, targets)
    in_maps = [{"x": xs[c], "idx": idx[c]} for c in range(NCORES)]
    return run_bass_kernel_spmd(
        _get_nc(), in_maps, core_ids=list(range(NCORES)), **kwargs
    )


def kernel(output, targets):
    res = _run(output, targets)
    total = sum(float(r["partial"][0, 0]) for r in res.results)
    return np.array(np.float32(1.0) - np.float32(total / N), dtype=np.float32)



# revision 2
# speedup vs baseline: 1.4432x; 1.4432x over previous
"""Trainium2 Bass kernel for nn_CosineLoss: mean_i(1 - output[i, targets[i]]).

Strategy (data-parallel over the batch dim, 8 cores):
  - Core c owns rows [c*1024, (c+1)*1024) of `output` ([1024, 32000] f32 shard)
    plus flat element offsets idx[i] = i*32000 + targets[i] for its rows
    (int32, laid out [128, 8] in SBUF; descriptor address math in the SWDGE
    is integer, while on-device ALU adds go through an fp32 path that
    corrupts indices above 2^24).
  - On device: 8 indirect DMAs (128 descriptors each -- the HW unrolls one
    descriptor per dest partition row, so 128 scattered elements per
    instruction is the cap) gather the 1024 needed f32 elements from HBM
    (4 KB instead of 131 MB).
  - Raw bass (no TileContext): a hand-rolled semaphore graph avoids the
    tile-pool exit cleanup (drain + RANGE_CLEAR + barrier sandwich, ~2us of
    the measured window).
  - Split free-dim reduce 7/1: cols 0:7 reduce while gather 8 is still in
    flight; only a [128,1] add sits behind the last gather's completion.
    A [128,1] matmul against the framework's const-1.0 tile collapses
    partitions; the Activation engine copies the [1,1] PSUM scalar to SBUF
    and triggers the output DMA in engine order (no cross-engine hop).
  - Host sums the 8 partials and returns 1 - total/8192 as a () f32 array.
"""

import numpy as np

from concourse import bacc, bass, mybir
from concourse.bass_utils import run_bass_kernel_spmd

N = 8192
C = 32000
NCORES = 8
NL = N // NCORES  # 1024 rows per core
P = 128
F = NL // P  # 8 gathered elements per partition

_NC_CACHE = {}


def _build():
    # Bacc (not Bass): its compile() runs generate_event_semaphores, which
    # splits multi-sem waits -- walrus codegen allows 1 sync wait per inst.
    nc = bacc.Bacc("TRN2")
    x = nc.dram_tensor("x", [NL, C], mybir.dt.float32, kind="ExternalInput")
    idx = nc.dram_tensor("idx", [P, F], mybir.dt.int32, kind="ExternalInput")
    partial = nc.dram_tensor("partial", [1, 1], mybir.dt.float32, kind="ExternalOutput")

    idx_t = nc.alloc_sbuf_tensor("idx_t", [P, F], mybir.dt.int32)
    gath = nc.alloc_sbuf_tensor("gath", [P, F], mybir.dt.float32)
    red_a = nc.alloc_sbuf_tensor("red_a", [P, 1], mybir.dt.float32)
    red_f = nc.alloc_sbuf_tensor("red_f", [P, 1], mybir.dt.float32)
    res = nc.alloc_sbuf_tensor("res", [1, 1], mybir.dt.float32)
    acc = nc.alloc_psum_tensor("acc", [1, 1], mybir.dt.float32)

    # ones for the partition-reduce matmul: the framework preamble already
    # memsets a [128,1] f32 1.0 const tile, and every engine passes the
    # init barrier after it -- no extra memset or semaphore needed.
    ones = nc.const_aps.aps[(mybir.dt.float32, 1.0)]

    s_idx = nc.alloc_semaphore("s_idx")  # idx DMA completion (+16)
    s_ga = nc.alloc_semaphore("s_ga")  # gathers 1-7 DMA completions (+16 each)
    s_gb = nc.alloc_semaphore("s_gb")  # gather 8 DMA completion (+16)
    s_v = nc.alloc_semaphore("s_v")  # DVE progress
    s_mm = nc.alloc_semaphore("s_mm")  # matmul (HIGH pass) done
    s_out = nc.alloc_semaphore("s_out")  # walrus requires an update on every DMA

    nc.sync.dma_start(out=idx_t.ap(), in_=idx[:]).then_inc(s_idx, 16)

    # dummy early copy on the (idle) Activation engine: if walrus needs an
    # activation-table load for `copy`, it lands here, fully overlapped with
    # the idx DMA, instead of on the critical path before the real copy.
    actwarm = nc.alloc_sbuf_tensor("actwarm", [1, 1], mybir.dt.float32)
    nc.scalar.copy(out=actwarm.ap(), in_=nc.const_aps.aps[(mybir.dt.float32, 0.0)][0:1, :])

    # (A dummy SWDGE warmup gather was tried here to hide the first indirect
    # DMA's ~0.9us Q7 IRAM load behind the idx DMA; measured 24.6us vs 24.2us
    # without -- the ucode load only starts when the first DMA_INDIRECT
    # executes, so a warmup just pays it earlier plus its own gen time.)

    # ONE indirect DMA carrying all 1024 offsets ([128,8] dest + [128,8]
    # offset AP): the interp maps each offset element to one gathered
    # element (num_elem_per_idx = out.size // idx.size = 1). Q7 descriptor
    # gen is the serial bottleneck (994ns fixed + 0.34ns/desc), so one
    # instruction (~1.4us) replaces 8 (~11us).
    nc.gpsimd.wait_ge(s_idx, 16)
    nc.gpsimd.indirect_dma_start(
        out=gath.ap(),
        out_offset=None,
        in_=x[:],
        in_offset=bass.IndirectOffsetOnAxis(ap=idx_t.ap(), axis=1),
    ).then_inc(s_gb, 16)

    nc.vector.wait_ge(s_gb, 16)
    nc.vector.tensor_reduce(
        out=red_f.ap(),
        in_=gath.ap(),
        axis=mybir.AxisListType.X,
        op=mybir.AluOpType.add,
    ).then_inc(s_v, 1)

    # partition-reduce via matmul with ones as lhsT: LDWEIGHTS(ones)
    # prefetches early (move_matmul_waits_to_ldweights hoists the waits);
    # only the rhs stream (red_f) sits on the critical path.
    nc.tensor.wait_ge(s_v, 1)
    nc.tensor.matmul(
        out=acc.ap(), lhsT=ones, rhs=red_f.ap(), start=True, stop=True
    ).then_inc(s_mm, 1)

    # PSUM->SBUF copy and the output DMA both on the Activation engine
    # (an HWDGE engine): engine-queue order replaces the copy->DMA
    # cross-engine semaphore hop.
    nc.scalar.wait_ge(s_mm, 1)
    nc.scalar.copy(out=res.ap(), in_=acc.ap())
    nc.scalar.dma_start(out=partial[:], in_=res.ap()).then_inc(s_out, 16)

    nc.compile()
    return nc


def _get_nc():
    if "nc" not in _NC_CACHE:
        _NC_CACHE["nc"] = _build()
    return _NC_CACHE["nc"]


def _shard(output, targets):
    xs = np.ascontiguousarray(
        output.reshape(NCORES, NL, C).astype(np.float32, copy=False)
    )
    flat = np.arange(NL, dtype=np.int32) * C + targets.reshape(NCORES, NL).astype(
        np.int32
    )
    return xs, np.ascontiguousarray(flat.reshape(NCORES, P, F))


def _run(output, targets, **kwargs):
    xs, idx = _shard(output, targets)
    in_maps = [{"x": xs[c], "idx": idx[c]} for c in range(NCORES)]
    return run_bass_kernel_spmd(
        _get_nc(), in_maps, core_ids=list(range(NCORES)), **kwargs
    )


def kernel(output, targets):
    res = _run(output, targets)
    total = sum(float(r["partial"][0, 0]) for r in res.results)
    return np.array(np.float32(1.0) - np.float32(total / N), dtype=np.float32)

